# revision 12
# baseline (speedup 1.0000x reference)
"""LogSpaceTripleRCell Trainium2 kernel.

Strategy: data-parallel over batch B=8 across 8 NeuronCores (1 sample/core,
no collectives). Per core, everything is SBUF-resident:
  phase 1: A/C = x @ {R_x, W_delta/2}.T + {b, b_delta/2}   (big fp16 matmuls)
  loop:    1024 sequential steps, weight-stationary fp16 matmuls (N=1 moving),
           delta-gate computed via tanh only: sigmoid(z) = (tanh(z/2)+1)/2
           with the 0.5 folded into the delta-side weights on host.
  phase 3: out = softmax_groups(h) * silu(h @ W_out.T), log|h|, sign(h),
           h transposed back to [t, i] via PE transposes.
Host does: spectral norm, weight transposes/casts, final assembly.
"""
import sys, os
sys.path.insert(0, '/opt/trn_rl_repo')
import numpy as np

import concourse.bass as bass
import concourse.bacc as bacc
import concourse.tile as tile
from concourse import mybir, bass_utils
from concourse.alu_op_type import AluOpType
from concourse.masks import make_identity

F32, F16 = mybir.dt.float32, mybir.dt.float16
AF = mybir.ActivationFunctionType
T, B, D = 1024, 8, 1024
G, GS = 32, 32

_CACHE = {}


def _spectral_norm(W, u0, target_radius=0.99):
    W = W.astype(np.float32)
    u = u0 / (np.linalg.norm(u0) + 1e-8)
    v = None
    for _ in range(3):
        v = W.T @ u
        v = v / (np.linalg.norm(v) + 1e-8)
        u = W @ v
        u = u / (np.linalg.norm(u) + 1e-8)
    sigma = np.abs(u @ (W @ v))
    return W * (target_radius / (sigma + 1e-8))


def _build():
    nc = bacc.Bacc("TRN2", target_bir_lowering=False, debug=False)
    wl_d = nc.dram_tensor("wl", [D, 2 * D], F16, kind="ExternalInput")
    wx_d = nc.dram_tensor("wx", [D, 2 * D], F16, kind="ExternalInput")
    wxl_d = nc.dram_tensor("wxl", [D, 2 * D], F16, kind="ExternalInput")
    xTl_d = nc.dram_tensor("xTl", [D, T], F16, kind="ExternalInput")
    wo_d = nc.dram_tensor("wo", [D, D], F16, kind="ExternalInput")
    xT_d = nc.dram_tensor("xT", [D, T], F16, kind="ExternalInput")
    bias_d = nc.dram_tensor("bias", [128, 16], F32, kind="ExternalInput")
    h0_d = nc.dram_tensor("h0", [128, 8], F32, kind="ExternalInput")
    out_d = nc.dram_tensor("out_o", [T, D], F32, kind="ExternalOutput")
    log_d = nc.dram_tensor("log_o", [T, D], F32, kind="ExternalOutput")
    sgn_d = nc.dram_tensor("sgn_o", [T, D], F32, kind="ExternalOutput")
    hli_d = nc.dram_tensor("hli_o", [T, D], F32, kind="ExternalOutput")

    with tile.TileContext(nc) as tc:
        with tc.tile_pool(name="main", bufs=1) as mp:
            W_loop = mp.tile([128, 8, 2 * D], F16)
            nc.default_dma_engine.dma_start(
                W_loop[:], wl_d[:].rearrange("(k p) i -> p k i", p=128))
            AC = mp.tile([128, 16, T], F32)
            bias_sb = mp.tile([128, 16], F32)
            nc.default_dma_engine.dma_start(bias_sb[:], bias_d[:])
            h_cur = mp.tile([128, 8], F32)
            nc.default_dma_engine.dma_start(h_cur[:], h0_d[:])
            h16 = mp.tile([128, 8], F16)
            nc.vector.tensor_copy(h16[:], h_cur[:])

            # ---------------- phase 1: AC = x @ wx + bias ----------------
            with tc.tile_pool(name="ph1", bufs=1) as p1, \
                 tc.tile_pool(name="ph1ps", bufs=2, space="PSUM") as p1ps:
                xT = p1.tile([128, 8, T], F16, tag="xh")
                nc.default_dma_engine.dma_start(
                    xT[:], xT_d[:].rearrange("(k p) t -> p k t", p=128))
                xTl = p1.tile([128, 8, T], F16, tag="xl")
                nc.default_dma_engine.dma_start(
                    xTl[:], xTl_d[:].rearrange("(k p) t -> p k t", p=128))
                # pass A+C with W_x_hi stationary: (Wh xh + Wh xl) + bias
                W_x = p1.tile([128, 8, 2 * D], F16, tag="wx")
                nc.default_dma_engine.dma_start(
                    W_x[:], wx_d[:].rearrange("(k p) i -> p k i", p=128))
                for m in range(16):
                    for c2 in range(2):
                        ps = p1ps.tile([128, 512], F32)
                        for k in range(8):
                            nc.tensor.matmul(
                                ps[:], W_x[:, k, 128 * m:128 * (m + 1)],
                                xT[:, k, 512 * c2:512 * (c2 + 1)],
                                start=(k == 0), stop=False)
                        for k in range(8):
                            nc.tensor.matmul(
                                ps[:], W_x[:, k, 128 * m:128 * (m + 1)],
                                xTl[:, k, 512 * c2:512 * (c2 + 1)],
                                start=False, stop=(k == 7))
                        nc.scalar.activation(
                            AC[:, m, 512 * c2:512 * (c2 + 1)], ps[:],
                            AF.Identity, bias=bias_sb[:, m:m + 1])
                # pass B: AC += Wl xh
                W_xl = p1.tile([128, 8, 2 * D], F16, tag="wx")
                nc.default_dma_engine.dma_start(
                    W_xl[:], wxl_d[:].rearrange("(k p) i -> p k i", p=128))
                for m in range(16):
                    for c2 in range(2):
                        ps = p1ps.tile([128, 512], F32)
                        for k in range(8):
                            nc.tensor.matmul(
                                ps[:], W_xl[:, k, 128 * m:128 * (m + 1)],
                                xT[:, k, 512 * c2:512 * (c2 + 1)],
                                start=(k == 0), stop=(k == 7))
                        acs = AC[:, m, 512 * c2:512 * (c2 + 1)]
                        nc.vector.tensor_tensor(acs, acs, ps[:],
                                                AluOpType.add)

            # ---------------- sequential recurrence ----------------
            hp_ctx = tc.tile_pool(name="histp", bufs=1)
            hp = hp_ctx.__enter__()
            hist = hp.tile([128, 8, T], F16)
            H = hp.tile([128, 8, T], F32)
            with tc.tile_pool(name="lps", bufs=2, space="PSUM") as lps, \
                 tc.tile_pool(name="chain", bufs=2) as cp:
                with tc.For_i(0, T, staggered_reset=True) as t:
                    ps = lps.tile([128, 16], F32)
                    for m in range(16):
                        for k in range(8):
                            nc.tensor.matmul(
                                ps[:, m:m + 1],
                                W_loop[:, k, 128 * m:128 * (m + 1)],
                                h16[:, k:k + 1], start=(k == 0), stop=(k == 7))
                    v = cp.tile([128, 16], F32)
                    nc.vector.tensor_tensor(
                        v[:], ps[:], AC[:, :, bass.ds(t, 1)].squeeze(),
                        AluOpType.add)
                    s = cp.tile([128, 16], F32)
                    nc.scalar.activation(s[:], v[:], AF.Tanh)
                    diff = cp.tile([128, 8], F32)
                    nc.vector.tensor_tensor(diff[:], s[:, 0:8], h_cur[:],
                                            AluOpType.subtract)
                    g2 = cp.tile([128, 8], F32)
                    nc.vector.scalar_tensor_tensor(
                        g2[:], s[:, 8:16], 1.0, diff[:],
                        AluOpType.add, AluOpType.mult)
                    nc.vector.scalar_tensor_tensor(
                        h_cur[:], g2[:], 0.5, h_cur[:],
                        AluOpType.mult, AluOpType.add)
                    nc.vector.tensor_copy(H[:, :, bass.ds(t, 1)].squeeze(),
                                          h_cur[:])
                    nc.vector.tensor_copy(h16[:], h_cur[:])
                    nc.vector.tensor_copy(
                        hist[:, :, bass.ds(t, 1)].squeeze(), h16[:])

            # ---------------- phase 3: outputs ----------------
            with tc.tile_pool(name="ph3", bufs=1) as p3, \
                 tc.tile_pool(name="ph3c", bufs=1) as p3c, \
                 tc.tile_pool(name="ph3ps", bufs=2, space="PSUM") as p3ps, \
                 tc.tile_pool(name="trps", bufs=2, space="PSUM") as trps:
                W_out = p3c.tile([128, 8, D], F16)
                nc.default_dma_engine.dma_start(
                    W_out[:], wo_d[:].rearrange("(k p) i -> p k i", p=128))
                ident = p3c.tile([128, 128], F32)
                make_identity(nc, ident)
                eps = p3c.tile([128, 1], F32)
                nc.vector.memset(eps[:], 1e-12)
                for tt in range(8):
                    t0 = 128 * tt
                    sil = p3.tile([128, D], F32, tag="sil")
                    for c2 in range(2):
                        pso = p3ps.tile([128, 512], F32)
                        for k in range(8):
                            nc.tensor.matmul(
                                pso[:], hist[:, k, t0:t0 + 128],
                                W_out[:, k, 512 * c2:512 * (c2 + 1)],
                                start=(k == 0), stop=(k == 7))
                        nc.scalar.activation(sil[:, 512 * c2:512 * (c2 + 1)],
                                             pso[:], AF.Silu)
                    hl = p3.tile([128, D], F32, tag="hl")
                    for k in range(8):
                        tp = trps.tile([128, 128], F32)
                        nc.tensor.transpose(tp[:], H[:, k, t0:t0 + 128],
                                            ident[:])
                        nc.vector.tensor_copy(hl[:, 128 * k:128 * (k + 1)],
                                              tp[:])
                    nc.default_dma_engine.dma_start(hli_d[t0:t0 + 128, :],
                                                    hl[:])
                    # log|h| and sign(h)
                    ha = p3.tile([128, D], F32, tag="ha")
                    nc.scalar.activation(ha[:], hl[:], AF.Abs)
                    lg = p3.tile([128, D], F32, tag="lg")
                    nc.scalar.activation(lg[:], ha[:], AF.Ln, bias=eps[:])
                    nc.default_dma_engine.dma_start(log_d[t0:t0 + 128, :],
                                                    lg[:])
                    sg = p3.tile([128, D], F32, tag="sg")
                    nc.vector.tensor_scalar(sg[:], hl[:], 0.0, None,
                                            AluOpType.is_ge)
                    nc.vector.tensor_scalar(sg[:], sg[:], 2.0, -1.0,
                                            AluOpType.mult, AluOpType.add)
                    nc.default_dma_engine.dma_start(sgn_d[t0:t0 + 128, :],
                                                    sg[:])
                    # grouped softmax * silu
                    hg = hl[:].rearrange("p (g s) -> p g s", s=GS)
                    gmax = p3.tile([128, G], F32, tag="gmax")
                    nc.vector.tensor_reduce(gmax[:], hg,
                                            mybir.AxisListType.X,
                                            AluOpType.max)
                    ex = p3.tile([128, D], F32, tag="ex")
                    exg = ex[:].rearrange("p (g s) -> p g s", s=GS)
                    nc.vector.tensor_tensor(
                        exg, hg, gmax[:].unsqueeze(-1).broadcast_to(
                            [128, G, GS]), AluOpType.subtract)
                    nc.scalar.activation(ex[:], ex[:], AF.Exp)
                    gsum = p3.tile([128, G], F32, tag="gsum")
                    nc.vector.tensor_reduce(gsum[:], exg,
                                            mybir.AxisListType.X,
                                            AluOpType.add)
                    rcp = p3.tile([128, G], F32, tag="rcp")
                    nc.vector.reciprocal(rcp[:], gsum[:])
                    nc.vector.tensor_tensor(
                        exg, exg, rcp[:].unsqueeze(-1).broadcast_to(
                            [128, G, GS]), AluOpType.mult)
                    ot = p3.tile([128, D], F32, tag="ot")
                    nc.vector.tensor_tensor(ot[:], ex[:], sil[:],
                                            AluOpType.mult)
                    nc.default_dma_engine.dma_start(out_d[t0:t0 + 128, :],
                                                    ot[:])
            hp_ctx.__exit__(None, None, None)
    nc.compile()
    return nc


def kernel(x, log_h0, sign_h0, R_h_raw, R_x, R_delta_raw, W_delta, W_out,
           b, b_delta, u_h, u_delta, n_groups):
    x = np.asarray(x, np.float32)
    log_h0 = np.asarray(log_h0, np.float32)
    sign_h0 = np.asarray(sign_h0, np.float32)
    assert int(n_groups) == 32 and x.shape == (T, B, D)

    R_h = _spectral_norm(np.asarray(R_h_raw, np.float32),
                         np.asarray(u_h, np.float32))
    R_d = _spectral_norm(np.asarray(R_delta_raw, np.float32),
                         np.asarray(u_delta, np.float32))
    R_x_ = np.asarray(R_x, np.float32)
    W_d = np.asarray(W_delta, np.float32)
    W_o = np.asarray(W_out, np.float32)
    b_ = np.asarray(b, np.float32)
    bd_ = np.asarray(b_delta, np.float32)

    wl = np.concatenate([R_h.T, 0.5 * R_d.T], axis=1).astype(np.float16)
    wx32 = np.concatenate([R_x_.T, 0.5 * W_d.T], axis=1)
    wx = wx32.astype(np.float16)
    wxl = (wx32 - wx.astype(np.float32)).astype(np.float16)
    wo = np.ascontiguousarray(W_o.T).astype(np.float16)
    bias = np.concatenate([b_.reshape(8, 128).T,
                           0.5 * bd_.reshape(8, 128).T], axis=1)
    bias = np.ascontiguousarray(bias, np.float32)
    with np.errstate(under='ignore', over='ignore'):
        h0 = sign_h0 * np.exp(log_h0)  # [B, D]

    if "nc" not in _CACHE:
        _CACHE["nc"] = _build()
    nc = _CACHE["nc"]

    in_maps = []
    for c in range(B):
        xT32 = np.ascontiguousarray(x[:, c, :].T)
        xT = xT32.astype(np.float16)
        xTl = (xT32 - xT.astype(np.float32)).astype(np.float16)
        in_maps.append({
            "wl": wl, "wx": wx, "wxl": wxl, "wo": wo, "bias": bias,
            "xT": xT, "xTl": xTl,
            "h0": np.ascontiguousarray(h0[c].reshape(8, 128).T, np.float32),
        })
    res = bass_utils.run_bass_kernel_spmd(nc, in_maps,
                                          core_ids=list(range(8)))

    out = np.empty((T, B, D), np.float32)
    log_h = np.empty((T + 1, B, D), np.float32)
    sign_h = np.empty((T + 1, B, D), np.float32)
    h_lin = np.empty((T, B, D), np.float32)
    log_h[0], sign_h[0] = log_h0, sign_h0
    for c in range(B):
        r = res.results[c]
        out[:, c, :] = r["out_o"]
        log_h[1:, c, :] = r["log_o"]
        sign_h[1:, c, :] = r["sgn_o"]
        h_lin[:, c, :] = r["hli_o"]
    return out, log_h, sign_h, h_lin
